# revision 1
# baseline (speedup 1.0000x reference)
"""Causal multi-head attention (B=4, S=2048, D=1024, H=16) on 8 TRN2 NeuronCores.

Sharding: 4 batches x 2 head-groups (8 heads each) -> 8 cores.
Each core:
  - projects its batch's tokens through its head-group's Wq/Wk/Wv columns,
    directly in transposed [head_dim, token] layout so the QK^T and PV
    matmuls need no on-device transposes,
  - computes causal attention (mask = tril(k=1): one future token allowed)
    for its 8 heads; scoresT blocks [k,q] are exponentiated on the scalar
    engine and multiplied by {0,1} masks on the vector engine; softmax
    denominators come from a ones-column appended to V so the PV matmul
    accumulates both ctx^T and the exp-sums,
  - computes the partial output projection ctx_part @ Wo[group rows] + bo/2,
  - ReduceScatter(add) over the 2 cores of each batch leaves each core
    holding half the tokens of its batch; the host concatenates.

All matmuls run as float32r (TF32-like; full PE rate at moving dim 512).
"""

import numpy as np

B, S, D = 4, 2048, 1024
H = 16
HD = D // H  # 64
G = 2  # head groups (tensor-parallel degree per batch)
HPG = H // G  # 8 heads per core
DG = D // G  # 512 dims per group
P = 128
NKT = D // P  # 8 k-tiles over d_model
NQC = S // 512  # 4 query chunks of 512
NTT = S // P  # 16 token tiles of 128
NR = DG // P  # 4 dim-tiles (head pairs) per group

_CACHE = {}


def _build_masks():
    """masks[s] is the [128, 512] multiplicative mask for a scoresT block
    [k_local, q_chunk_local] whose k-block index is kb = 4*qc + s.
    Allowed iff global k <= global q + 1."""
    masks = np.zeros((5, P, 512), dtype=np.float32)
    i = np.arange(P)[:, None]  # k local
    jj = np.arange(P)[None, :]  # q local within 128-subblock
    for s in range(5):
        for j in range(4):  # q subblock within the 512 chunk
            blk = masks[s][:, 128 * j : 128 * (j + 1)]
            if j > s:
                blk[:] = 1.0
            elif j == s:
                blk[:] = (i <= jj + 1).astype(np.float32)
            elif j == s - 1:
                blk[0, 127] = 1.0
    return masks


def _build_bass(collective=True):
    import concourse.bacc as bacc
    import concourse.mybir as mybir
    import concourse.tile as tile

    f32 = mybir.dt.float32
    f32r = mybir.dt.float32r
    AF = mybir.ActivationFunctionType

    nc = bacc.Bacc("TRN2", target_bir_lowering=False, debug=False, num_devices=8)

    xT = nc.dram_tensor("xT", [D, S], f32r, kind="ExternalInput").ap()
    wq = nc.dram_tensor("wq", [D, DG], f32r, kind="ExternalInput").ap()
    wk = nc.dram_tensor("wk", [D, DG], f32r, kind="ExternalInput").ap()
    wv = nc.dram_tensor("wv", [D, DG], f32r, kind="ExternalInput").ap()
    wo = nc.dram_tensor("wo", [DG, D], f32r, kind="ExternalInput").ap()
    bo_b = nc.dram_tensor("bo_b", [P, D], f32, kind="ExternalInput").ap()
    masks = nc.dram_tensor("masks", [5, P, 512], f32r, kind="ExternalInput").ap()
    out_ext = nc.dram_tensor("out", [S // 2, D], f32, kind="ExternalOutput").ap()

    with tile.TileContext(nc) as tc:
        with (
            tc.tile_pool(name="pqk", bufs=1) as pqk,
            tc.tile_pool(name="pv", bufs=1) as pv,
            tc.tile_pool(name="pmask", bufs=1) as pmask,
            tc.tile_pool(name="pdram", bufs=1, space="DRAM") as pdram,
        ):
            # persistent SBUF tensors
            qT_sb = pqk.tile([P, NR, S], f32r)  # [dims of pair r | token]
            kT_sb = pqk.tile([P, NR, S], f32r)
            va_sb = pv.tile([P, NTT, HPG, HD + 1], f32r)  # v + ones col
            masks_sb = pmask.tile([P, 5, 512], f32r)
            nc.sync.dma_start(masks_sb[:], masks.rearrange("s p q -> p s q"))
            # ones column of va: masks[s=0] block j=3 is all 1.0 (j > s), and
            # memset can't encode an f32r immediate, so copy ones from there.
            nc.vector.tensor_copy(
                va_sb[:, :, :, HD : HD + 1],
                masks_sb[:, 0, 384:512].rearrange("p (a b) -> p a b", b=HPG)[
                    :, :, :, None
                ],
            )

            partial = pdram.tile([S, D], f32)
            rs_out = pdram.tile([S // 2, D], f32)

            # ---------------- projections ----------------
            with (
                tc.tile_pool(name="pw", bufs=3) as pw,
                tc.tile_pool(name="px", bufs=2) as px,
                tc.tile_pool(name="pp", bufs=2, space="PSUM") as pp,
            ):
                w_sbs = {}
                for name, w in (("wq", wq), ("wk", wk), ("wv", wv)):
                    w_sb = pw.tile([P, NKT, DG], f32r, name=f"w_{name}", tag="w")
                    nc.sync.dma_start(w_sb[:], w.rearrange("(ko p) f -> p ko f", p=P))
                    w_sbs[name] = w_sb

                xT_r = xT.rearrange("(ko p) t -> p ko t", p=P)
                for t in range(NQC):
                    tok = slice(512 * t, 512 * (t + 1))
                    xtile = px.tile([P, NKT, 512], f32r, name="xtile", tag="x")
                    nc.sync.dma_start(xtile[:], xT_r[:, :, tok])
                    # qT / kT: out [dims(pair r), 512 tokens]
                    for name, dst in (("wq", qT_sb), ("wk", kT_sb)):
                        w_sb = w_sbs[name]
                        for rr in range(NR):
                            ps = pp.tile([P, 512], f32, name="ps_proj", tag="ps")
                            for kt in range(NKT):
                                nc.tensor.matmul(
                                    ps[:],
                                    w_sb[:, kt, P * rr : P * (rr + 1)],
                                    xtile[:, kt, :],
                                    start=(kt == 0),
                                    stop=(kt == NKT - 1),
                                )
                            nc.vector.tensor_copy(dst[:, rr, tok], ps[:])
                    # v: out [128 tokens, 512 dims] per token tile
                    w_sb = w_sbs["wv"]
                    for st in range(4):
                        tt = 4 * t + st
                        ps = pp.tile([P, 512], f32, name="ps_v", tag="ps")
                        for kt in range(NKT):
                            nc.tensor.matmul(
                                ps[:],
                                xtile[:, kt, 128 * st : 128 * (st + 1)],
                                w_sb[:, kt, :],
                                start=(kt == 0),
                                stop=(kt == NKT - 1),
                            )
                        nc.vector.tensor_copy(
                            va_sb[:, tt, :, 0:HD],
                            ps[:].rearrange("p (h d) -> p h d", d=HD),
                        )

            # ---------------- attention + output projection ----------------
            with (
                tc.tile_pool(name="pw2", bufs=1) as pw2,
                tc.tile_pool(name="pc", bufs=1) as pc,
                tc.tile_pool(name="pe", bufs=2) as pe,
                tc.tile_pool(name="pn", bufs=2) as pn,
                tc.tile_pool(name="po_sb", bufs=2) as po_sb,
                tc.tile_pool(name="psS", bufs=2, space="PSUM") as psS,
                tc.tile_pool(name="psC", bufs=2, space="PSUM") as psC,
            ):
                ctxT_sb = pc.tile([P, NR, S], f32r)
                wo_sb = pw2.tile([P, NR, D], f32r)
                nc.sync.dma_start(wo_sb[:], wo.rearrange("(ko p) f -> p ko f", p=P))
                bo_sb = pw2.tile([P, D], f32)
                nc.sync.dma_start(bo_sb[:], bo_b[:])

                for pr in range(NR):
                    for qc in range(NQC):
                        qs = slice(512 * qc, 512 * (qc + 1))
                        nkb = min(4 * qc + 5, NTT)
                        ctxs = [
                            psC.tile([HD + 1, 512], f32, name=f"ctx{hl}", tag=f"ctx{hl}")
                            for hl in range(2)
                        ]
                        # matmul operands must sit at base partition 0 on this
                        # HW path, so the odd head's qT/kT slices (partitions
                        # 64:128) are staged through base-0 copies.
                        qTs = pn.tile([64, 512], f32r, name="qTs", tag="qTs")
                        nc.vector.tensor_copy(qTs[:], qT_sb[64:P, pr, qs])
                        for kb in range(nkb):
                            ks = slice(128 * kb, 128 * (kb + 1))
                            kTs = pn.tile([64, 128], f32r, name="kTs", tag="kTs", bufs=3)
                            nc.vector.tensor_copy(kTs[:], kT_sb[64:P, pr, ks])
                            sc = psS.tile([P, 1024], f32, name="sc", tag="sc")
                            nc.tensor.matmul(
                                sc[:, 0:512],
                                kT_sb[0:64, pr, ks],
                                qT_sb[0:64, pr, qs],
                                start=True,
                                stop=True,
                            )
                            nc.tensor.matmul(
                                sc[:, 512:1024],
                                kTs[:],
                                qTs[:],
                                start=True,
                                stop=True,
                            )
                            et = pe.tile([P, 1024], f32r, name="et", tag="et")
                            nc.scalar.activation(et[:], sc[:], AF.Exp, scale=1.0 / 8.0)
                            s = kb - 4 * qc
                            if 0 <= s <= 4:
                                for hl in range(2):
                                    nc.vector.tensor_mul(
                                        et[:, 512 * hl : 512 * (hl + 1)],
                                        et[:, 512 * hl : 512 * (hl + 1)],
                                        masks_sb[:, s, :],
                                    )
                            for hl in range(2):
                                nc.tensor.matmul(
                                    ctxs[hl][:],
                                    va_sb[:, kb, 2 * pr + hl, :],
                                    et[:, 512 * hl : 512 * (hl + 1)],
                                    start=(kb == 0),
                                    stop=(kb == nkb - 1),
                                )
                        # normalize: ctxT_h = ctx[0:64] * (1 / ctx[64]) -> SBUF
                        for hl in range(2):
                            ctx = ctxs[hl]
                            srow = pn.tile([1, 512], f32, name="srow", tag="srow")
                            nc.vector.tensor_copy(srow[:], ctx[HD : HD + 1, :])
                            # stage ctx out of PSUM right away so the bank is
                            # released before the DRAM-broadcast round trip
                            stage = pn.tile([64, 512], f32, name="stage", tag="stage")
                            nc.vector.tensor_copy(stage[:], ctx[0:HD, :])
                            srow_d = pdram.tile(
                                [1, 512], f32, name="srow_d", tag="srow_d", bufs=6
                            )
                            nc.sync.dma_start(srow_d[:], srow[:])
                            bc = pn.tile([64, 512], f32, name="bc", tag="bc")
                            nc.sync.dma_start(
                                bc[:], srow_d[0:1, :].to_broadcast((64, 512))
                            )
                            rc = pn.tile([64, 512], f32, name="rc", tag="rc")
                            nc.vector.reciprocal(rc[:], bc[:])
                            nc.vector.tensor_mul(
                                ctxT_sb[64 * hl : 64 * (hl + 1), pr, qs],
                                stage[:],
                                rc[:],
                            )

                # output projection: partial = ctx_part @ Wo_part + bo/2
                for tt in range(NTT):
                    ts_ = slice(128 * tt, 128 * (tt + 1))
                    for nch in range(2):
                        ns = slice(512 * nch, 512 * (nch + 1))
                        # share the score pool's 2-bank slots (bank budget:
                        # psS 4 + psC 2x2 = 8)
                        ps = psS.tile([P, 512], f32, name="ps_o", tag="sc")
                        for rr in range(NR):
                            nc.tensor.matmul(
                                ps[:],
                                ctxT_sb[:, rr, ts_],
                                wo_sb[:, rr, ns],
                                start=(rr == 0),
                                stop=(rr == NR - 1),
                            )
                        ot = po_sb.tile([P, 512], f32, name="ot", tag="ot")
                        nc.vector.tensor_add(ot[:], ps[:], bo_sb[:, ns])
                        nc.sync.dma_start(partial[ts_, ns], ot[:])

                if collective:
                    nc.gpsimd.collective_compute(
                        "ReduceScatter",
                        mybir.AluOpType.add,
                        replica_groups=[[0, 1], [2, 3], [4, 5], [6, 7]],
                        ins=[partial.opt()],
                        outs=[rs_out.opt()],
                    )
                    nc.sync.dma_start(out_ext[:], rs_out[:])
                else:
                    nc.sync.dma_start(out_ext[:], partial[0 : S // 2, :])

    nc.compile()
    return nc


def _in_maps(x, Wq, Wk, Wv, Wo, bo):
    masks = _build_masks()
    maps = []
    for c in range(8):
        b, g = c // 2, c % 2
        cols = slice(DG * g, DG * (g + 1))
        maps.append(
            {
                "xT": np.ascontiguousarray(np.asarray(x)[b].T, dtype=np.float32),
                "wq": np.ascontiguousarray(np.asarray(Wq)[:, cols], dtype=np.float32),
                "wk": np.ascontiguousarray(np.asarray(Wk)[:, cols], dtype=np.float32),
                "wv": np.ascontiguousarray(np.asarray(Wv)[:, cols], dtype=np.float32),
                "wo": np.ascontiguousarray(np.asarray(Wo)[cols, :], dtype=np.float32),
                "bo_b": np.broadcast_to(
                    np.asarray(bo, dtype=np.float32) / G, (P, D)
                ).copy(),
                "masks": masks,
            }
        )
    return maps


def _get_nc():
    if "nc" not in _CACHE:
        _CACHE["nc"] = _build_bass()
    return _CACHE["nc"]


def run(inputs, trace=False):
    from concourse.bass_utils import run_bass_kernel_spmd

    nc = _get_nc()
    maps = _in_maps(**inputs)
    res = run_bass_kernel_spmd(nc, maps, list(range(8)), trace=trace)
    out = np.empty((B, S, D), dtype=np.float32)
    for c in range(8):
        b, g = c // 2, c % 2
        out[b, g * (S // 2) : (g + 1) * (S // 2), :] = res.results[c]["out"]
    return out, res


def kernel(x, Wq, Wk, Wv, Wo, bo):
    out, _ = run(dict(x=x, Wq=Wq, Wk=Wk, Wv=Wv, Wo=Wo, bo=bo))
    return out



# revision 2
# speedup vs baseline: 1.4254x; 1.4254x over previous
"""Causal multi-head attention (B=4, S=2048, D=1024, H=16) on 8 TRN2 NeuronCores.

Sharding: 4 batches x 2 head-groups (8 heads each) -> 8 cores.
Each core:
  - projects its batch's tokens through its head-group's Wq/Wk/Wv columns,
    directly in transposed [head_dim, token] layout so the QK^T and PV
    matmuls need no on-device transposes,
  - computes causal attention (mask = tril(k=1): one future token allowed)
    for its 8 heads. Score matmuls for the two heads of a pair run as a
    row-tiled concurrent pair on the PE (head A rows 0:64, head B rows
    64:128), keeping the full 128x128 array active so the HAM clock-gate
    stays at 8/8. Causal masking is an additive -1e9 accumulated into the
    score PSUM via an identity-stationary matmul; fully-masked column
    ranges are skipped entirely (scores, exp and PV all narrow near the
    diagonal). exp runs on the scalar engine writing bf16 probs; the PV
    matmuls use a packed [vaA|1|vaB|1] stationary whose ones columns
    accumulate the softmax denominators in the same PSUM tiles,
  - normalizes via a [1,512] reciprocal + DRAM-broadcast + multiply,
  - per 512-token q-chunk: output projection ctx_part @ Wo[group rows]
    + bo/2, then a chunked ReduceScatter(add) over the 2 cores of each
    batch so the collective overlaps the next chunk's attention.

All f32 matmuls run as float32r (TF32-like; full PE rate); probs are bf16.
"""

import numpy as np

B, S, D = 4, 2048, 1024
H = 16
HD = D // H  # 64
G = 2  # head groups (tensor-parallel degree per batch)
HPG = H // G  # 8 heads per core
DG = D // G  # 512 dims per group
P = 128
NKT = D // P  # 8 k-tiles over d_model
NQC = S // 512  # 4 query chunks of 512
NTT = S // P  # 16 token tiles of 128
NR = DG // P  # 4 dim-tiles (head pairs) per group
NEG = -1.0e9

_CACHE = {}


def _build_masks():
    """masks[s] is the [128, 256] additive mask for the partially-masked
    column window of a scoresT block [k_local, q] with s = kb - 4*qc >= 0.
    Layout: cols 0:128 = subblock j=s-1 (all NEG except the corner element
    [0,127] which is 0), cols 128:256 = subblock j=s (0 where k <= q+1 else
    NEG). s=0 uses only cols 128:256 (the triangle); s=4 only cols 0:128."""
    masks = np.full((5, P, 256), NEG, dtype=np.float32)
    i = np.arange(P)[:, None]
    jj = np.arange(P)[None, :]
    for s in range(5):
        masks[s][:, 0:128] = NEG
        masks[s][0, 127] = 0.0  # corner: k=0 vs q=last of subblock j=s-1
        masks[s][:, 128:256] = np.where(i <= jj + 1, 0.0, NEG)
    return masks


def _mask_window(s):
    """(psum col offset, width, mask source col offset) for state s."""
    if s == 0:
        return 0, 128, 128
    if s == 4:
        return 384, 128, 0
    return 128 * (s - 1), 256, 0


def _build_bass(collective=True):
    import concourse.bacc as bacc
    import concourse.mybir as mybir
    import concourse.tile as tile

    f32 = mybir.dt.float32
    f32r = mybir.dt.float32r
    bf16 = mybir.dt.bfloat16
    AF = mybir.ActivationFunctionType

    nc = bacc.Bacc("TRN2", target_bir_lowering=False, debug=False, num_devices=8)

    xT = nc.dram_tensor("xT", [D, S], f32r, kind="ExternalInput").ap()
    wq = nc.dram_tensor("wq", [D, DG], f32r, kind="ExternalInput").ap()
    wk = nc.dram_tensor("wk", [D, DG], f32r, kind="ExternalInput").ap()
    wv = nc.dram_tensor("wv", [D, DG], f32r, kind="ExternalInput").ap()
    wo = nc.dram_tensor("wo", [DG, D], f32r, kind="ExternalInput").ap()
    bo_b = nc.dram_tensor("bo_b", [P, D], f32, kind="ExternalInput").ap()
    masks = nc.dram_tensor("masks", [5, P, 256], f32r, kind="ExternalInput").ap()
    ident = nc.dram_tensor("ident", [P, P], f32r, kind="ExternalInput").ap()
    out_ext = nc.dram_tensor("out", [S // 2, D], f32, kind="ExternalOutput").ap()

    with tile.TileContext(nc) as tc:
        with (
            tc.tile_pool(name="pqk", bufs=1) as pqk,
            tc.tile_pool(name="pv", bufs=1) as pv,
            tc.tile_pool(name="pmask", bufs=1) as pmask,
            tc.tile_pool(name="pw2", bufs=1) as pw2,
            tc.tile_pool(name="pdram", bufs=1, space="DRAM") as pdram,
        ):
            # persistent SBUF tensors
            qT_sb = pqk.tile([P, NR, S], f32r)  # [dims of pair r | token]
            kT_sb = pqk.tile([P, NR, S], f32r)
            # packed V per head pair: [vaA(64) | 1 | vaB(64) | 1] = 130 cols
            va_sb = pv.tile([P, NTT, NR, 130], bf16)
            masks_sb = pmask.tile([P, 5, 256], f32r)
            ident_sb = pmask.tile([P, P], f32r)
            nc.sync.dma_start(masks_sb[:], masks.rearrange("s p q -> p s q"))
            nc.sync.dma_start(ident_sb[:], ident)
            nc.vector.memset(va_sb[:, :, :, 64:65], 1.0)
            nc.vector.memset(va_sb[:, :, :, 129:130], 1.0)
            # wo/bo loaded up front so the DMA overlaps the projections
            wo_sb = pw2.tile([P, NR, D], f32r)
            nc.sync.dma_start(wo_sb[:], wo.rearrange("(ko p) f -> p ko f", p=P))
            bo_sb = pw2.tile([P, D], f32)
            nc.sync.dma_start(bo_sb[:], bo_b[:])

            partial = pdram.tile([S, D], f32)
            rs_out = pdram.tile([S // 2, D], f32)

            # ---------------- projections ----------------
            with (
                tc.tile_pool(name="pw", bufs=3) as pw,
                tc.tile_pool(name="px", bufs=2) as px,
                tc.tile_pool(name="pp", bufs=2, space="PSUM") as pp,
            ):
                w_sbs = {}
                xT_r = xT.rearrange("(ko p) t -> p ko t", p=P)
                xtiles = []
                # interleave weight/x DMAs so the first matmuls start early
                for name, w in (("wq", wq), ("wk", wk), ("wv", wv)):
                    w_sb = pw.tile([P, NKT, DG], f32r, name=f"w_{name}", tag="w")
                    nc.sync.dma_start(w_sb[:], w.rearrange("(ko p) f -> p ko f", p=P))
                    w_sbs[name] = w_sb
                    if name == "wq":
                        xt = px.tile([P, NKT, 512], f32r, name="xtile", tag="x")
                        nc.sync.dma_start(xt[:], xT_r[:, :, 0:512])
                        xtiles.append(xt)

                for t in range(NQC):
                    tok = slice(512 * t, 512 * (t + 1))
                    xtile = xtiles[t]
                    if t + 1 < NQC:
                        xt = px.tile([P, NKT, 512], f32r, name="xtile", tag="x")
                        nc.sync.dma_start(
                            xt[:], xT_r[:, :, 512 * (t + 1) : 512 * (t + 2)]
                        )
                        xtiles.append(xt)
                    # qT / kT: out [dims(pair r), 512 tokens]
                    for name, dst in (("wq", qT_sb), ("wk", kT_sb)):
                        w_sb = w_sbs[name]
                        for rr in range(NR):
                            ps = pp.tile([P, 512], f32, name="ps_proj", tag="ps")
                            for kt in range(NKT):
                                nc.tensor.matmul(
                                    ps[:],
                                    w_sb[:, kt, P * rr : P * (rr + 1)],
                                    xtile[:, kt, :],
                                    start=(kt == 0),
                                    stop=(kt == NKT - 1),
                                )
                            nc.vector.tensor_copy(dst[:, rr, tok], ps[:])
                    # v: out [128 tokens, 512 dims] per token tile, split into
                    # the pair-packed [vaA|1|vaB|1] bf16 layout
                    w_sb = w_sbs["wv"]
                    for st in range(4):
                        tt = 4 * t + st
                        ps = pp.tile([P, 512], f32, name="ps_v", tag="ps")
                        for kt in range(NKT):
                            nc.tensor.matmul(
                                ps[:],
                                xtile[:, kt, 128 * st : 128 * (st + 1)],
                                w_sb[:, kt, :],
                                start=(kt == 0),
                                stop=(kt == NKT - 1),
                            )
                        pshd = ps[:].rearrange("p (r two d) -> p r two d", two=2, d=HD)
                        nc.vector.tensor_copy(va_sb[:, tt, :, 0:HD], pshd[:, :, 0, :])
                        nc.vector.tensor_copy(
                            va_sb[:, tt, :, 65 : 65 + HD], pshd[:, :, 1, :]
                        )

            # ---------------- attention + output projection ----------------
            with (
                tc.tile_pool(name="pc", bufs=1) as pc,
                tc.tile_pool(name="pe", bufs=6) as pe,
                tc.tile_pool(name="pn", bufs=2) as pn,
                tc.tile_pool(name="po_sb", bufs=2) as po_sb,
                tc.tile_pool(name="psS", bufs=2, space="PSUM") as psS,
                tc.tile_pool(name="psC", bufs=2, space="PSUM") as psC,
            ):
                ctxT_sb = pc.tile([P, NR, S], f32r)

                for qc in range(NQC):
                    qs = slice(512 * qc, 512 * (qc + 1))
                    nkb = min(4 * qc + 5, NTT)
                    for pr in range(NR):
                        ctxA = psC.tile([P, 512], f32, name="ctxA", tag="ctxA")
                        ctxB = psC.tile([65, 512], f32, name="ctxB", tag="ctxB")
                        for kb in range(nkb):
                            ks = slice(128 * kb, 128 * (kb + 1))
                            s = kb - 4 * qc
                            c0 = max(0, 128 * (s - 1)) if s > 0 else 0
                            w = 512 - c0
                            qsn = slice(512 * qc + c0, 512 * (qc + 1))
                            sc = psS.tile([P, 1024], f32, name="sc", tag="sc")
                            # row-tiled concurrent pair: head A rows 0:64,
                            # head B rows 64:128 of the PE array
                            masked = 0 <= s <= 4
                            nc.tensor.matmul(
                                sc[:, c0:512],
                                kT_sb[0:64, pr, ks],
                                qT_sb[0:64, pr, qsn],
                                start=True,
                                stop=not masked,
                                skip_group_check=True,
                            )
                            nc.tensor.matmul(
                                sc[:, 512 + c0 : 1024],
                                kT_sb[64:P, pr, ks],
                                qT_sb[64:P, pr, qsn],
                                start=True,
                                stop=not masked,
                                skip_group_check=True,
                            )
                            if masked:
                                m0, mw, ms0 = _mask_window(s)
                                for hl in range(2):
                                    nc.tensor.matmul(
                                        sc[:, 512 * hl + m0 : 512 * hl + m0 + mw],
                                        ident_sb[:],
                                        masks_sb[:, s, ms0 : ms0 + mw],
                                        start=False,
                                        stop=True,
                                        skip_group_check=True,
                                    )
                            et = pe.tile([P, 2, 512], bf16, name="et", tag="et")
                            scv = sc[:].rearrange("p (h q) -> p h q", h=2)
                            nc.scalar.activation(
                                et[:, :, c0:512],
                                scv[:, :, c0:512],
                                AF.Exp,
                                scale=1.0 / 8.0,
                            )
                            nc.tensor.matmul(
                                ctxA[:, c0:512],
                                va_sb[:, kb, pr, 0:128],
                                et[:, 0, c0:512],
                                start=(kb == 0),
                                stop=(kb == nkb - 1),
                                skip_group_check=True,
                            )
                            nc.tensor.matmul(
                                ctxB[:, c0:512],
                                va_sb[:, kb, pr, 65:130],
                                et[:, 1, c0:512],
                                start=(kb == 0),
                                stop=(kb == nkb - 1),
                                skip_group_check=True,
                            )
                        # normalize: ctxT_h = ctx[0:64] * (1 / sums) -> SBUF
                        for hl, ctx in ((0, ctxA), (1, ctxB)):
                            srow = pn.tile([1, 512], f32, name="srow", tag="srow")
                            nc.vector.reciprocal(srow[:], ctx[HD : HD + 1, :])
                            srow_d = pdram.tile(
                                [1, 512], f32, name="srow_d", tag="srow_d", bufs=6
                            )
                            nc.sync.dma_start(srow_d[:], srow[:])
                            bc = pn.tile([64, 512], f32, name="bc", tag="bc", bufs=4)
                            nc.sync.dma_start(
                                bc[:], srow_d[0:1, :].to_broadcast((64, 512))
                            )
                            nc.vector.tensor_mul(
                                ctxT_sb[64 * hl : 64 * (hl + 1), pr, qs],
                                ctx[0:HD, :],
                                bc[:],
                            )

                    # output projection for this q-chunk:
                    # partial = ctx_part @ Wo_part + bo/2
                    for st in range(4):
                        tt = 4 * qc + st
                        ts_ = slice(128 * tt, 128 * (tt + 1))
                        for nch in range(2):
                            ns = slice(512 * nch, 512 * (nch + 1))
                            # share the score pool's 2-bank slots
                            ps = psS.tile([P, 512], f32, name="ps_o", tag="sc")
                            for rr in range(NR):
                                nc.tensor.matmul(
                                    ps[:],
                                    ctxT_sb[:, rr, ts_],
                                    wo_sb[:, rr, ns],
                                    start=(rr == 0),
                                    stop=(rr == NR - 1),
                                )
                            ot = po_sb.tile([P, 512], f32, name="ot", tag="ot")
                            nc.vector.tensor_add(ot[:], ps[:], bo_sb[:, ns])
                            nc.sync.dma_start(partial[ts_, ns], ot[:])

                    # chunked ReduceScatter: overlaps the next chunk's attention
                    if collective:
                        nc.gpsimd.collective_compute(
                            "ReduceScatter",
                            mybir.AluOpType.add,
                            replica_groups=[[0, 1], [2, 3], [4, 5], [6, 7]],
                            ins=[partial[qs, :].opt()],
                            outs=[rs_out[256 * qc : 256 * (qc + 1), :].opt()],
                        )
                        nc.sync.dma_start(
                            out_ext[256 * qc : 256 * (qc + 1), :],
                            rs_out[256 * qc : 256 * (qc + 1), :],
                        )
                    else:
                        nc.sync.dma_start(
                            out_ext[256 * qc : 256 * (qc + 1), :],
                            partial[512 * qc : 512 * qc + 256, :],
                        )

    nc.compile()
    return nc


def _in_maps(x, Wq, Wk, Wv, Wo, bo):
    masks = _build_masks()
    ident = np.eye(P, dtype=np.float32)
    maps = []
    for c in range(8):
        b, g = c // 2, c % 2
        cols = slice(DG * g, DG * (g + 1))
        maps.append(
            {
                "xT": np.ascontiguousarray(np.asarray(x)[b].T, dtype=np.float32),
                "wq": np.ascontiguousarray(np.asarray(Wq)[:, cols], dtype=np.float32),
                "wk": np.ascontiguousarray(np.asarray(Wk)[:, cols], dtype=np.float32),
                "wv": np.ascontiguousarray(np.asarray(Wv)[:, cols], dtype=np.float32),
                "wo": np.ascontiguousarray(np.asarray(Wo)[cols, :], dtype=np.float32),
                "bo_b": np.broadcast_to(
                    np.asarray(bo, dtype=np.float32) / G, (P, D)
                ).copy(),
                "masks": masks,
                "ident": ident,
            }
        )
    return maps


def _get_nc():
    if "nc" not in _CACHE:
        _CACHE["nc"] = _build_bass()
    return _CACHE["nc"]


def run(inputs, trace=False):
    from concourse.bass_utils import run_bass_kernel_spmd

    nc = _get_nc()
    maps = _in_maps(**inputs)
    res = run_bass_kernel_spmd(nc, maps, list(range(8)), trace=trace)
    out = np.empty((B, S, D), dtype=np.float32)
    for c in range(8):
        b, g = c // 2, c % 2
        ro = res.results[c]["out"]
        for qc in range(NQC):
            out[b, 512 * qc + 256 * g : 512 * qc + 256 * (g + 1), :] = ro[
                256 * qc : 256 * (qc + 1)
            ]
    return out, res


def kernel(x, Wq, Wk, Wv, Wo, bo):
    out, _ = run(dict(x=x, Wq=Wq, Wk=Wk, Wv=Wv, Wo=Wo, bo=bo))
    return out


# revision 9
# speedup vs baseline: 1.6019x; 1.1238x over previous
"""Causal multi-head attention (B=4, S=2048, D=1024, H=16) on 8 TRN2 NeuronCores.

Sharding: 4 batches x 2 head-groups (8 heads each) -> 8 cores.
Each core:
  - projects its batch's tokens through its head-group's Wq/Wk/Wv columns,
    directly in transposed [head_dim, token] layout so the QK^T and PV
    matmuls need no on-device transposes,
  - computes causal attention (mask = tril(k=1): one future token allowed)
    for its 8 heads. Score matmuls for the two heads of a pair run as a
    row-tiled concurrent pair on the PE (head A rows 0:64, head B rows
    64:128), keeping the full 128x128 array active so the HAM clock-gate
    stays at 8/8. Causal masking is an additive -1e9 accumulated into the
    score PSUM via an identity-stationary matmul; fully-masked column
    ranges are skipped entirely (scores, exp and PV all narrow near the
    diagonal). exp runs on the scalar engine writing bf16 probs; the PV
    matmuls use a packed [vaA|1|vaB|1] stationary whose ones columns
    accumulate the softmax denominators in the same PSUM tiles,
  - normalizes via a [1,512] reciprocal + DRAM-broadcast + multiply,
  - per 512-token q-chunk: output projection ctx_part @ Wo[group rows]
    + bo/2, then a chunked ReduceScatter(add) over the 2 cores of each
    batch so the collective overlaps the next chunk's attention.

All f32 matmuls run as float32r (TF32-like; full PE rate); probs are bf16.
"""

import numpy as np

B, S, D = 4, 2048, 1024
H = 16
HD = D // H  # 64
G = 2  # head groups (tensor-parallel degree per batch)
HPG = H // G  # 8 heads per core
DG = D // G  # 512 dims per group
P = 128
NKT = D // P  # 8 k-tiles over d_model
NQC = S // 512  # 4 query chunks of 512
NTT = S // P  # 16 token tiles of 128
NR = DG // P  # 4 dim-tiles (head pairs) per group
NEG = -1.0e9

_CACHE = {}


def _build_masks():
    """masks[s] is the [128, 256] additive mask for the partially-masked
    column window of a scoresT block [k_local, q] with s = kb - 4*qc >= 0.
    Layout: cols 0:128 = subblock j=s-1 (all NEG except the corner element
    [0,127] which is 0), cols 128:256 = subblock j=s (0 where k <= q+1 else
    NEG). s=0 uses only cols 128:256 (the triangle); s=4 only cols 0:128."""
    masks = np.full((5, P, 256), NEG, dtype=np.float32)
    i = np.arange(P)[:, None]
    jj = np.arange(P)[None, :]
    for s in range(5):
        masks[s][:, 0:128] = NEG
        masks[s][0, 127] = 0.0  # corner: k=0 vs q=last of subblock j=s-1
        masks[s][:, 128:256] = np.where(i <= jj + 1, 0.0, NEG)
    return masks


def _mask_window(s):
    """(psum col offset, width, mask source col offset) for state s."""
    if s == 0:
        return 0, 128, 128
    if s == 4:
        return 384, 128, 0
    return 128 * (s - 1), 256, 0


def _build_bass(collective=True):
    import concourse.bacc as bacc
    import concourse.mybir as mybir
    import concourse.tile as tile

    f32 = mybir.dt.float32
    f32r = mybir.dt.float32r
    bf16 = mybir.dt.bfloat16
    AF = mybir.ActivationFunctionType

    nc = bacc.Bacc("TRN2", target_bir_lowering=False, debug=False, num_devices=8)

    xT = nc.dram_tensor("xT", [D, S], f32r, kind="ExternalInput").ap()
    wq = nc.dram_tensor("wq", [D, DG], f32r, kind="ExternalInput").ap()
    wk = nc.dram_tensor("wk", [D, DG], f32r, kind="ExternalInput").ap()
    wv = nc.dram_tensor("wv", [D, DG], f32r, kind="ExternalInput").ap()
    wo = nc.dram_tensor("wo", [DG, D], f32r, kind="ExternalInput").ap()
    bo_b = nc.dram_tensor("bo_b", [P, D], f32r, kind="ExternalInput").ap()
    masks = nc.dram_tensor("masks", [5, P, 256], f32r, kind="ExternalInput").ap()
    ident = nc.dram_tensor("ident", [P, P], f32r, kind="ExternalInput").ap()
    out_ext = nc.dram_tensor("out", [S // 2, D], f32, kind="ExternalOutput").ap()

    with tile.TileContext(nc) as tc:
        with (
            tc.tile_pool(name="pqk", bufs=1) as pqk,
            tc.tile_pool(name="pv", bufs=1) as pv,
            tc.tile_pool(name="pmask", bufs=1) as pmask,
            tc.tile_pool(name="pw2", bufs=1) as pw2,
            tc.tile_pool(name="pdram", bufs=1, space="DRAM") as pdram,
        ):
            # persistent SBUF tensors
            qT_sb = pqk.tile([P, NR, S], f32r)  # [dims of pair r | token]
            kT_sb = pqk.tile([P, NR, S], f32r)
            # packed V per head pair: [vaA(64) | 1 | vaB(64) | 1] = 130 cols
            va_sb = pv.tile([P, NTT, NR, 130], bf16)
            masks_sb = pmask.tile([P, 5, 256], f32r)
            ident_sb = pmask.tile([P, P], f32r)
            nc.sync.dma_start(masks_sb[:], masks.rearrange("s p q -> p s q"))
            nc.sync.dma_start(ident_sb[:], ident)
            nc.vector.memset(va_sb[:, :, :, 64:65], 1.0)
            nc.vector.memset(va_sb[:, :, :, 129:130], 1.0)
            # wo/bo loaded up front so the DMA overlaps the projections
            wo_sb = pw2.tile([P, NR, D], f32r)
            nc.sync.dma_start(wo_sb[:], wo.rearrange("(ko p) f -> p ko f", p=P))
            bo_sb = pw2.tile([P, D], f32r)
            nc.sync.dma_start(bo_sb[:], bo_b[:])

            partial = pdram.tile([S, D], f32)
            rs_out = pdram.tile([S // 2, D], f32)

            # ---------------- projections ----------------
            with (
                tc.tile_pool(name="pw", bufs=3) as pw,
                tc.tile_pool(name="px", bufs=2) as px,
                tc.tile_pool(name="pp", bufs=2, space="PSUM") as pp,
            ):
                w_sbs = {}
                xT_r = xT.rearrange("(ko p) t -> p ko t", p=P)
                xtiles = []
                # interleave weight/x DMAs so the first matmuls start early
                for name, w in (("wq", wq), ("wk", wk), ("wv", wv)):
                    w_sb = pw.tile([P, NKT, DG], f32r, name=f"w_{name}", tag="w")
                    nc.sync.dma_start(w_sb[:], w.rearrange("(ko p) f -> p ko f", p=P))
                    w_sbs[name] = w_sb
                    if name == "wq":
                        xt = px.tile([P, NKT, 512], f32r, name="xtile", tag="x")
                        nc.sync.dma_start(xt[:], xT_r[:, :, 0:512])
                        xtiles.append(xt)

                for t in range(NQC):
                    tok = slice(512 * t, 512 * (t + 1))
                    xtile = xtiles[t]
                    if t + 1 < NQC:
                        xt = px.tile([P, NKT, 512], f32r, name="xtile", tag="x")
                        nc.sync.dma_start(
                            xt[:], xT_r[:, :, 512 * (t + 1) : 512 * (t + 2)]
                        )
                        xtiles.append(xt)
                    # qT / kT: out [dims(pair r), 512 tokens]
                    for name, dst in (("wq", qT_sb), ("wk", kT_sb)):
                        w_sb = w_sbs[name]
                        for rr in range(NR):
                            ps = pp.tile([P, 512], f32, name="ps_proj", tag="ps")
                            for kt in range(NKT):
                                nc.tensor.matmul(
                                    ps[:],
                                    w_sb[:, kt, P * rr : P * (rr + 1)],
                                    xtile[:, kt, :],
                                    start=(kt == 0),
                                    stop=(kt == NKT - 1),
                                )
                            nc.vector.tensor_copy(dst[:, rr, tok], ps[:])
                    # v: out [128 tokens, 512 dims] per token tile, split into
                    # the pair-packed [vaA|1|vaB|1] bf16 layout
                    w_sb = w_sbs["wv"]
                    for st in range(4):
                        tt = 4 * t + st
                        ps = pp.tile([P, 512], f32, name="ps_v", tag="ps")
                        for kt in range(NKT):
                            nc.tensor.matmul(
                                ps[:],
                                xtile[:, kt, 128 * st : 128 * (st + 1)],
                                w_sb[:, kt, :],
                                start=(kt == 0),
                                stop=(kt == NKT - 1),
                            )
                        pshd = ps[:].rearrange("p (r two d) -> p r two d", two=2, d=HD)
                        nc.vector.tensor_copy(va_sb[:, tt, :, 0:HD], pshd[:, :, 0, :])
                        nc.vector.tensor_copy(
                            va_sb[:, tt, :, 65 : 65 + HD], pshd[:, :, 1, :]
                        )

            # ---------------- attention + output projection ----------------
            with (
                tc.tile_pool(name="pc", bufs=1) as pc,
                tc.tile_pool(name="pe", bufs=6) as pe,
                tc.tile_pool(name="pn", bufs=4) as pn,
                tc.tile_pool(name="po_sb", bufs=2) as po_sb,
                tc.tile_pool(name="psS", bufs=2, space="PSUM") as psS,
                tc.tile_pool(name="psC", bufs=2, space="PSUM") as psC,
            ):
                ctxT_sb = pc.tile([P, NR, S], f32r)

                def outproj(qc):
                    # partial = ctx_part @ Wo_part + bo/2 (bias via identity-MM)
                    for st in range(4):
                        tt = 4 * qc + st
                        ts_ = slice(128 * tt, 128 * (tt + 1))
                        for nch in range(2):
                            ns = slice(512 * nch, 512 * (nch + 1))
                            # share the score pool's 2-bank slots
                            ps = psS.tile([P, 512], f32, name="ps_o", tag="sc")
                            for rr in range(NR):
                                nc.tensor.matmul(
                                    ps[:],
                                    ctxT_sb[:, rr, ts_],
                                    wo_sb[:, rr, ns],
                                    start=(rr == 0),
                                    stop=False,
                                    skip_group_check=True,
                                )
                            nc.tensor.matmul(
                                ps[:],
                                ident_sb[:],
                                bo_sb[:, ns],
                                start=False,
                                stop=True,
                                skip_group_check=True,
                            )
                            ot = po_sb.tile([P, 512], f32, name="ot", tag="ot")
                            nc.vector.tensor_copy(ot[:], ps[:])
                            nc.sync.dma_start(partial[ts_, ns], ot[:])

                    # chunked ReduceScatter: overlaps the next chunk's attention
                    qs = slice(512 * qc, 512 * (qc + 1))
                    if collective:
                        nc.gpsimd.collective_compute(
                            "ReduceScatter",
                            mybir.AluOpType.add,
                            replica_groups=[[0, 1], [2, 3], [4, 5], [6, 7]],
                            ins=[partial[qs, :].opt()],
                            outs=[rs_out[256 * qc : 256 * (qc + 1), :].opt()],
                        )
                        nc.gpsimd.dma_start(
                            out_ext[256 * qc : 256 * (qc + 1), :],
                            rs_out[256 * qc : 256 * (qc + 1), :],
                        )
                    else:
                        nc.gpsimd.dma_start(
                            out_ext[256 * qc : 256 * (qc + 1), :],
                            partial[512 * qc : 512 * qc + 256, :],
                        )

                for qc in range(NQC):
                    qs = slice(512 * qc, 512 * (qc + 1))
                    nkb = min(4 * qc + 5, NTT)
                    for pr in range(NR):
                        ctxA = psC.tile([P, 512], f32, name="ctxA", tag="ctxA")
                        ctxB = psC.tile([65, 512], f32, name="ctxB", tag="ctxB")
                        pv_prev = None
                        for kb in range(nkb):
                            ks = slice(128 * kb, 128 * (kb + 1))
                            s = kb - 4 * qc
                            c0 = max(0, 128 * (s - 1)) if s > 0 else 0
                            qsn = slice(512 * qc + c0, 512 * (qc + 1))
                            sc = psS.tile([P, 1024], f32, name="sc", tag="sc")
                            # row-tiled concurrent pair: head A rows 0:64,
                            # head B rows 64:128 of the PE array
                            masked = 0 <= s <= 4
                            nc.tensor.matmul(
                                sc[:, c0:512],
                                kT_sb[0:64, pr, ks],
                                qT_sb[0:64, pr, qsn],
                                start=True,
                                stop=not masked,
                                skip_group_check=True,
                            )
                            nc.tensor.matmul(
                                sc[:, 512 + c0 : 1024],
                                kT_sb[64:P, pr, ks],
                                qT_sb[64:P, pr, qsn],
                                start=True,
                                stop=not masked,
                                skip_group_check=True,
                            )
                            if masked:
                                m0, mw, ms0 = _mask_window(s)
                                for hl in range(2):
                                    nc.tensor.matmul(
                                        sc[:, 512 * hl + m0 : 512 * hl + m0 + mw],
                                        ident_sb[:],
                                        masks_sb[:, s, ms0 : ms0 + mw],
                                        start=False,
                                        stop=True,
                                        skip_group_check=True,
                                    )
                            # PV of the previous block: fills the PE while the
                            # scalar engine computes this block's exp
                            if pv_prev is not None:
                                pc0, pet, pkb = pv_prev
                                nc.tensor.matmul(
                                    ctxA[:, pc0:512],
                                    va_sb[:, pkb, pr, 0:128],
                                    pet[:, 0, pc0:512],
                                    start=(pkb == 0),
                                    stop=False,
                                    skip_group_check=True,
                                )
                                nc.tensor.matmul(
                                    ctxB[:, pc0:512],
                                    va_sb[:, pkb, pr, 65:130],
                                    pet[:, 1, pc0:512],
                                    start=(pkb == 0),
                                    stop=False,
                                    skip_group_check=True,
                                )
                            et = pe.tile([P, 2, 512], bf16, name="et", tag="et")
                            scv = sc[:].rearrange("p (h q) -> p h q", h=2)
                            nc.scalar.activation(
                                et[:, :, c0:512],
                                scv[:, :, c0:512],
                                AF.Exp,
                                scale=1.0 / 8.0,
                            )
                            pv_prev = (c0, et, kb)
                        pc0, pet, pkb = pv_prev
                        nc.tensor.matmul(
                            ctxA[:, pc0:512],
                            va_sb[:, pkb, pr, 0:128],
                            pet[:, 0, pc0:512],
                            start=(pkb == 0),
                            stop=True,
                            skip_group_check=True,
                        )
                        nc.tensor.matmul(
                            ctxB[:, pc0:512],
                            va_sb[:, pkb, pr, 65:130],
                            pet[:, 1, pc0:512],
                            start=(pkb == 0),
                            stop=True,
                            skip_group_check=True,
                        )
                        # normalize: ctxT_h = ctx[0:64] * (1 / sums) -> SBUF.
                        # stage-copy on gpsimd releases the PSUM bank early;
                        # reciprocal_approx_fast is ~5x faster than reciprocal.
                        for hl, ctx in ((0, ctxA), (1, ctxB)):
                            # sums row to a base-0 tile (recip_approx_fast
                            # misbehaves on base-partition-64 inputs)
                            srow0 = pn.tile([1, 512], f32, name="srow0", tag="srow0")
                            nc.vector.tensor_copy(srow0[:], ctx[HD : HD + 1, :])
                            srow = pn.tile([1, 512], f32, name="srow", tag="srow")
                            nc.vector.reciprocal_approx_fast(srow[:], srow0[:])
                            srow_d = pdram.tile(
                                [1, 512], f32, name="srow_d", tag="srow_d", bufs=8
                            )
                            nc.sync.dma_start(srow_d[:], srow[:])
                            bc = pn.tile([64, 512], f32, name="bc", tag="bc")
                            nc.sync.dma_start(
                                bc[:], srow_d[0:1, :].to_broadcast((64, 512))
                            )
                            nc.vector.tensor_mul(
                                ctxT_sb[64 * hl : 64 * (hl + 1), pr, qs],
                                ctx[0:HD, :],
                                bc[:],
                            )
                    # software pipeline: emit the previous chunk's output
                    # projection + collective after this chunk's attention, so
                    # the PE has independent work while normalizations drain
                    if qc > 0:
                        outproj(qc - 1)
                outproj(NQC - 1)

    nc.compile()
    return nc


def _in_maps(x, Wq, Wk, Wv, Wo, bo):
    masks = _build_masks()
    ident = np.eye(P, dtype=np.float32)
    maps = []
    for c in range(8):
        b, g = c // 2, c % 2
        cols = slice(DG * g, DG * (g + 1))
        maps.append(
            {
                "xT": np.ascontiguousarray(np.asarray(x)[b].T, dtype=np.float32),
                "wq": np.ascontiguousarray(np.asarray(Wq)[:, cols], dtype=np.float32),
                "wk": np.ascontiguousarray(np.asarray(Wk)[:, cols], dtype=np.float32),
                "wv": np.ascontiguousarray(np.asarray(Wv)[:, cols], dtype=np.float32),
                "wo": np.ascontiguousarray(np.asarray(Wo)[cols, :], dtype=np.float32),
                "bo_b": np.broadcast_to(
                    np.asarray(bo, dtype=np.float32) / G, (P, D)
                ).copy(),
                "masks": masks,
                "ident": ident,
            }
        )
    return maps


def _get_nc():
    if "nc" not in _CACHE:
        _CACHE["nc"] = _build_bass()
    return _CACHE["nc"]


def run(inputs, trace=False):
    from concourse.bass_utils import run_bass_kernel_spmd

    nc = _get_nc()
    maps = _in_maps(**inputs)
    res = run_bass_kernel_spmd(nc, maps, list(range(8)), trace=trace)
    out = np.empty((B, S, D), dtype=np.float32)
    for c in range(8):
        b, g = c // 2, c % 2
        ro = res.results[c]["out"]
        for qc in range(NQC):
            out[b, 512 * qc + 256 * g : 512 * qc + 256 * (g + 1), :] = ro[
                256 * qc : 256 * (qc + 1)
            ]
    return out, res


def kernel(x, Wq, Wk, Wv, Wo, bo):
    out, _ = run(dict(x=x, Wq=Wq, Wk=Wk, Wv=Wv, Wo=Wo, bo=bo))
    return out
